# revision 2
# baseline (speedup 1.0000x reference)
"""Trainium2 Bass kernel for nn_ATTLSTMcell (B=32, T=512, H=1024, N=512).

Strategy: 8-way model parallelism over the gate output dimension (each core
owns 128 output columns of each of the 4 gates; the batch and the attention
are replicated). The r-readout is algebraically folded away:
    r_{t-1} @ Wr = softmax(h_{t-1} @ memory^T) @ (memory @ Wr)
so each step runs: AllGather(h^T slice) -> scores -> softmax -> gates ->
c/h update. The x @ Wx contribution and the bias are extra matmuls
accumulated into the same PSUM group (they do not depend on h, so they
execute during the AllGather window). Matmuls run in float32r (full-rate
single-pass fp32 on the PE).
"""
import sys
sys.path.insert(0, '/opt/trn_rl_repo')

import numpy as np
import concourse.bass as bass
import concourse.bacc as bacc
import concourse.mybir as mybir
import concourse.tile as tile
from concourse.bass import ds
from concourse.bass_utils import run_bass_kernel_spmd
from concourse.masks import make_identity

dt = mybir.dt
F32 = dt.float32
F32R = dt.float32r
F16 = dt.float16
AF = mybir.ActivationFunctionType
ALU = mybir.AluOpType

B = 32
H = 1024
N_MEM = 512
N_CORES = 8
SL = 512          # per-core gate-output slice (4 gates x 128)
KT = H // 128     # 8 k-tiles over H
T_FULL = 512

_cache = {}


def _build(T: int):
    nc = bacc.Bacc("TRN2", target_bir_lowering=False, debug=False,
                   num_devices=N_CORES)

    def inp(name, shape, dtype):
        return nc.dram_tensor(name, list(shape), dtype, kind="ExternalInput")

    wh = inp("wh", [KT, 128, SL], F32R)
    wr = inp("wr", [KT, 128, SL], F32R)
    wx = inp("wx", [KT, 128, SL], F32R)
    memt = inp("memt", [KT, 128, N_MEM], F32R)
    xt_in = inp("xt", [T, KT, 128, B], F32R)
    xres = inp("xres", [T, B, 128], F32)
    biasv = inp("biasv", [1, SL], F32R)
    ones_in = inp("ones", [1, B], F32R)
    h0t = inp("h0t", [KT, 128, B], F32R)
    r0t = inp("r0t", [KT, 128, B], F32R)
    c0 = inp("c0", [B, 128], F32)

    hout = nc.dram_tensor("hout", [T, B, 128], F32, kind="ExternalOutput")

    with tile.TileContext(nc) as tc:
        with tc.tile_pool(name="const", bufs=1) as cpool, \
             tc.tile_pool(name="wpool", bufs=1) as wpool:
            wh_sb = wpool.tile([128, KT, SL], F32R)
            nc.sync.dma_start(wh_sb[:], wh.ap().rearrange("k p n -> p k n"))
            memt_sb = wpool.tile([128, KT, N_MEM], F32R)
            nc.sync.dma_start(memt_sb[:], memt.ap().rearrange("k p n -> p k n"))
            wr_sb = wpool.tile([128, KT, SL], F32R)
            nc.sync.dma_start(wr_sb[:], wr.ap().rearrange("k p n -> p k n"))
            wx_sb = wpool.tile([128, KT, SL], F32R)
            nc.sync.dma_start(wx_sb[:], wx.ap().rearrange("k p n -> p k n"))
            mr_sb = wpool.tile([128, 4, SL], F32R)  # M_r = memory @ Wr
            biasv_sb = cpool.tile([1, SL], F32R)
            nc.sync.dma_start(biasv_sb[:], biasv.ap())
            ones_sb = cpool.tile([1, B], F32R)
            nc.sync.dma_start(ones_sb[:], ones_in.ap())
            ident_f = cpool.tile([32, 32], F32)
            make_identity(nc, ident_f[:])
            ident_r = cpool.tile([32, 32], F32R)
            nc.vector.tensor_copy(ident_r[:], ident_f[:])
            c_st = cpool.tile([B, 128], F32)
            nc.sync.dma_start(c_st[:], c0.ap())
            r0t_sb = cpool.tile([128, KT, B], F32R)
            nc.sync.dma_start(r0t_sb[:], r0t.ap().rearrange("k p b -> p k b"))

            # P0: M_r = memory @ Wr  (mem-rows on partitions)
            with tc.tile_pool(name="p0ps", bufs=2, space="PSUM") as p0ps:
                for m in range(4):
                    ps = p0ps.tile([128, SL], F32, tag="mr")
                    for k in range(KT):
                        nc.tensor.matmul(ps[:], memt_sb[:, k, ds(m * 128, 128)],
                                         wr_sb[:, k, :],
                                         start=(k == 0), stop=(k == KT - 1))
                    nc.vector.tensor_copy(mr_sb[:, m, :], ps[:])

            # P2: the recurrence
            with tc.tile_pool(name="loop", bufs=3) as pool, \
                 tc.tile_pool(name="small", bufs=4) as sm, \
                 tc.tile_pool(name="cstate", bufs=2) as cpool2, \
                 tc.tile_pool(name="psS", bufs=2, space="PSUM") as psS, \
                 tc.tile_pool(name="psG", bufs=4, space="PSUM") as psG, \
                 tc.tile_pool(name="psB", bufs=1, space="PSUM") as psB, \
                 tc.tile_pool(name="dram", bufs=3, space="DRAM") as dram:

                gat = pool.tile([128, KT, B], F32R, tag="gat")
                nc.sync.dma_start(gat[:], h0t.ap().rearrange("k p b -> p k b"))

                def open_gates_psum(t):
                    """x-part + bias of step t's gate pre-activations; no
                    dependence on h_{t-1} -> fills the AG idle window."""
                    xts = pool.tile([128, KT, B], F32R, tag="xts")
                    nc.sync.dma_start(xts[:],
                                      xt_in.ap()[t].rearrange("k p b -> p k b"))
                    ps_g = psG.tile([B, SL], F32, tag="g")
                    for k in range(KT):
                        nc.tensor.matmul(ps_g[:], xts[:, k, :], wx_sb[:, k, :],
                                         start=(k == 0), stop=False)
                    nc.tensor.matmul(ps_g[:], ones_sb[:], biasv_sb[:],
                                     start=False, stop=False)
                    return ps_g

                LOOKAHEAD = 3
                psg_pending = {}
                for tt in range(min(LOOKAHEAD, T)):
                    psg_pending[tt] = open_gates_psum(tt)

                for t in range(T):
                    ps_g = psg_pending.pop(t)
                    xr_t = pool.tile([B, 128], F32, tag="xrt")
                    nc.sync.dma_start(xr_t[:], xres.ap()[t])

                    # attention over h_{t-1} (t=0 uses the r0 input instead)
                    if t > 0:
                        ps_s = psS.tile([B, N_MEM], F32, tag="s")
                        for k in range(KT):
                            nc.tensor.matmul(ps_s[:], gat[:, k, :],
                                             memt_sb[:, k, :],
                                             start=(k == 0), stop=(k == KT - 1))
                        negmax = sm.tile([B, 1], F32, tag="negmax")
                        nc.vector.tensor_reduce(negmax[:], ps_s[:],
                                                axis=mybir.AxisListType.X,
                                                op=ALU.max, negate=True)
                        e_sb = pool.tile([B, N_MEM], F32, tag="e")
                        sumexp = sm.tile([B, 1], F32, tag="sumexp")
                        nc.scalar.activation(e_sb[:], ps_s[:], AF.Exp,
                                             bias=negmax[:], scale=1.0,
                                             accum_out=sumexp[:])
                        rec = sm.tile([B, 1], F32, tag="rec")
                        nc.vector.reciprocal(rec[:], sumexp[:])
                        e_r = pool.tile([B, N_MEM], F32R, tag="er")
                        nc.vector.tensor_scalar(e_r[:], e_sb[:], rec[:], None,
                                                op0=ALU.mult)
                        ps_e = psB.tile([128, 4, B], F32R, tag="et")
                        for j in range(4):
                            nc.tensor.transpose(ps_e[:, j, :],
                                                e_r[:, ds(j * 128, 128)],
                                                ident_r[:])
                        et_sb = pool.tile([128, 4, B], F32R, tag="ets")
                        nc.vector.tensor_copy(et_sb[:], ps_e[:])

                    for k in range(KT):
                        nc.tensor.matmul(ps_g[:], gat[:, k, :], wh_sb[:, k, :],
                                         start=False, stop=False)
                    if t == 0:
                        for k in range(KT):
                            nc.tensor.matmul(ps_g[:], r0t_sb[:, k, :],
                                             wr_sb[:, k, :],
                                             start=False, stop=(k == KT - 1))
                    else:
                        for j in range(4):
                            nc.tensor.matmul(ps_g[:], et_sb[:, j, :],
                                             mr_sb[:, j, :],
                                             start=False, stop=(j == 3))

                    # nonlinearities straight out of PSUM; col order [i|f|o|g]
                    iof = pool.tile([B, 384], F32, tag="iof")
                    nc.vector.tensor_scalar(iof[:], ps_g[:, 0:384], 0.0, 1.0,
                                            op0=ALU.max, op1=ALU.min)
                    gg = pool.tile([B, 128], F32, tag="gg")
                    nc.scalar.activation(gg[:], ps_g[:, ds(384, 128)], AF.Tanh)

                    ig = sm.tile([B, 128], F32, tag="ig")
                    nc.vector.tensor_tensor(ig[:], iof[:, 0:128], gg[:],
                                            op=ALU.mult)
                    c_new = cpool2.tile([B, 128], F32, tag="c")
                    nc.vector.tensor_tensor(c_new[:], iof[:, ds(128, 128)],
                                            c_st[:], op=ALU.mult)
                    nc.vector.tensor_tensor(c_new[:], c_new[:], ig[:],
                                            op=ALU.add)
                    c_st = c_new
                    h_t = pool.tile([B, 128], F32, tag="h")
                    nc.vector.tensor_tensor(h_t[:], iof[:, ds(256, 128)],
                                            c_new[:], op=ALU.mult)
                    nc.vector.tensor_tensor(h_t[:], h_t[:], xr_t[:],
                                            op=ALU.add)
                    nc.sync.dma_start(hout.ap()[t], h_t[:])

                    if t < T - 1:
                        ps_h = psB.tile([128, B], F32, tag="ht")
                        nc.tensor.transpose(ps_h[:], h_t[:], ident_f[:])
                        ht_sb = pool.tile([128, B], F16, tag="hts")
                        nc.vector.tensor_copy(ht_sb[:], ps_h[:])
                        ib = dram.tile([128, B], F16, tag="agin")
                        nc.sync.dma_start(ib[:], ht_sb[:])
                        ob = dram.tile([N_CORES * 128, B], F16, tag="agout")
                        nc.gpsimd.collective_compute(
                            "AllGather", ALU.bypass,
                            ins=[ib[:]], outs=[ob[:]],
                            replica_groups=[list(range(N_CORES))],
                        )
                        # x-part of step t+2 (2-step lookahead): sits before
                        # scores(t+1) in the static PE order, so it executes
                        # during the collective window.
                        if t + LOOKAHEAD < T:
                            psg_pending[t + LOOKAHEAD] = open_gates_psum(t + LOOKAHEAD)
                        gat_x = pool.tile([128, KT, B], F16, tag="gatx")
                        obv = ob.rearrange("(s p) b -> p s b", p=128)
                        nc.sync.dma_start(gat_x[:, 0:3, :], obv[:, 0:3, :])
                        nc.scalar.dma_start(gat_x[:, 3:6, :], obv[:, 3:6, :])
                        nc.gpsimd.dma_start(gat_x[:, 6:8, :], obv[:, 6:8, :])
                        gat = pool.tile([128, KT, B], F32R, tag="gat")
                        nc.vector.tensor_copy(gat[:], gat_x[:])

    nc.compile()
    return nc


def _prep_inputs(x, h0, r0, c0, memory, WI, bI, WF, bF, WC, bC, WO, bO, T):
    x = np.ascontiguousarray(np.asarray(x, np.float32)[:, :T])
    Ws = {"i": np.asarray(WI, np.float32), "f": np.asarray(WF, np.float32),
          "o": np.asarray(WO, np.float32), "g": np.asarray(WC, np.float32)}
    bs = {"i": np.asarray(bI, np.float32), "f": np.asarray(bF, np.float32),
          "o": np.asarray(bO, np.float32), "g": np.asarray(bC, np.float32)}
    memory = np.asarray(memory, np.float32)
    in_maps = []
    xt_full = np.ascontiguousarray(x.transpose(1, 2, 0).reshape(T, KT, 128, B))
    memt = memory.T.reshape(KT, 128, N_MEM).copy()
    h0t = np.asarray(h0, np.float32).T.reshape(KT, 128, B).copy()
    r0t = np.asarray(r0, np.float32).T.reshape(KT, 128, B).copy()
    ones = np.ones((1, B), np.float32)
    for c in range(N_CORES):
        sl = slice(c * 128, (c + 1) * 128)
        w_slices, b_parts = [], []
        for gname in ["i", "f", "o", "g"]:
            wg = Ws[gname][:, sl]
            bg = bs[gname][sl]
            if gname != "g":
                wg = wg * 0.2
                bg = bg * 0.2 + 0.5
            w_slices.append(wg)
            b_parts.append(bg)
        wfull = np.concatenate(w_slices, axis=1)  # [3072, 512]
        bias = np.concatenate(b_parts)
        in_maps.append({
            "wh": np.ascontiguousarray(wfull[0:H].reshape(KT, 128, SL)),
            "wr": np.ascontiguousarray(wfull[H:2 * H].reshape(KT, 128, SL)),
            "wx": np.ascontiguousarray(wfull[2 * H:3 * H].reshape(KT, 128, SL)),
            "memt": memt,
            "xt": xt_full,
            "xres": np.ascontiguousarray(x[:, :, sl].transpose(1, 0, 2)),
            "biasv": bias[None, :],
            "ones": ones,
            "h0t": h0t,
            "r0t": r0t,
            "c0": np.ascontiguousarray(np.asarray(c0, np.float32)[:, sl]),
        })
    return in_maps


def kernel(**inputs):
    T = int(inputs["x"].shape[1])
    if T not in _cache:
        _cache[T] = _build(T)
    nc = _cache[T]
    in_maps = _prep_inputs(T=T, **inputs)
    res = run_bass_kernel_spmd(nc, in_maps, core_ids=list(range(N_CORES)))
    hs = np.concatenate([res.results[c]["hout"] for c in range(N_CORES)],
                        axis=2)
    return np.ascontiguousarray(hs.transpose(1, 0, 2)).astype(np.float32)


# revision 3
# speedup vs baseline: 7.4609x; 7.4609x over previous
"""Trainium2 Bass kernel for nn_ATTLSTMcell (B=32, T=512, H=1024, N=512).

Strategy: 8-way model parallelism over the gate output dimension (each core
owns 128 output columns of each of the 4 gates; the batch and the attention
are replicated). The r-readout is algebraically folded away:
    r_{t-1} @ Wr = softmax(h_{t-1} @ memory^T) @ (memory @ Wr)
so each step runs: AllGather(h^T slice) -> scores -> softmax -> gates ->
c/h update. The x @ Wx contribution and the bias are extra matmuls
accumulated into the same PSUM group (they do not depend on h, so they
execute during the AllGather window, keeping the PE clock warm). Matmuls run
in float32r (full-rate single-pass fp32 on the PE; plain fp32 is 2x slower).
"""
import sys
sys.path.insert(0, '/opt/trn_rl_repo')

import numpy as np
import concourse.bass as bass
import concourse.bacc as bacc
import concourse.mybir as mybir
import concourse.tile as tile
from concourse.bass import ds
from concourse.bass_utils import run_bass_kernel_spmd
from concourse.masks import make_identity

dt = mybir.dt
F32 = dt.float32
F32R = dt.float32r
F16 = dt.float16
AF = mybir.ActivationFunctionType
ALU = mybir.AluOpType

B = 32
H = 1024
N_MEM = 512
N_CORES = 8
SL = 512          # per-core gate-output slice (4 gates x 128)
KT = H // 128     # 8 k-tiles over H
T_FULL = 512

_cache = {}


def _build(T: int):
    nc = bacc.Bacc("TRN2", target_bir_lowering=False, debug=False,
                   num_devices=N_CORES)

    def inp(name, shape, dtype):
        return nc.dram_tensor(name, list(shape), dtype, kind="ExternalInput")

    wh = inp("wh", [KT, 128, SL], F32R)
    wr = inp("wr", [KT, 128, SL], F32R)
    wx = inp("wx", [KT, 128, SL], F32R)
    memt = inp("memt", [KT, 128, N_MEM], F32R)
    xt_in = inp("xt", [T, KT, 128, B], F32R)
    xres = inp("xres", [T, B, 128], F32)
    biasv = inp("biasv", [1, SL], F32R)
    ones_in = inp("ones", [1, B], F32R)
    h0t = inp("h0t", [KT, 128, B], F32R)
    r0t = inp("r0t", [KT, 128, B], F32R)
    c0 = inp("c0", [B, 128], F32)

    hout = nc.dram_tensor("hout", [T, B, 128], F32, kind="ExternalOutput")

    with tile.TileContext(nc) as tc:
        with tc.tile_pool(name="const", bufs=1) as cpool, \
             tc.tile_pool(name="wpool", bufs=1) as wpool:
            wh_sb = wpool.tile([128, KT, SL], F32R)
            nc.sync.dma_start(wh_sb[:], wh.ap().rearrange("k p n -> p k n"))
            memt_sb = wpool.tile([128, KT, N_MEM], F32R)
            nc.sync.dma_start(memt_sb[:], memt.ap().rearrange("k p n -> p k n"))
            wr_sb = wpool.tile([128, KT, SL], F32R)
            nc.sync.dma_start(wr_sb[:], wr.ap().rearrange("k p n -> p k n"))
            wx_sb = wpool.tile([128, KT, SL], F32R)
            nc.sync.dma_start(wx_sb[:], wx.ap().rearrange("k p n -> p k n"))
            mr_sb = wpool.tile([128, 4, SL], F32R)  # M_r = memory @ Wr
            biasv_sb = cpool.tile([1, SL], F32R)
            nc.sync.dma_start(biasv_sb[:], biasv.ap())
            ones_sb = cpool.tile([1, B], F32R)
            nc.sync.dma_start(ones_sb[:], ones_in.ap())
            ident_f = cpool.tile([32, 32], F32)
            make_identity(nc, ident_f[:])
            ident_r = cpool.tile([32, 32], F32R)
            nc.vector.tensor_copy(ident_r[:], ident_f[:])
            c_st = cpool.tile([B, 128], F32)
            nc.sync.dma_start(c_st[:], c0.ap())
            r0t_sb = cpool.tile([128, KT, B], F32R)
            nc.sync.dma_start(r0t_sb[:], r0t.ap().rearrange("k p b -> p k b"))

            # P0: M_r = memory @ Wr  (mem-rows on partitions)
            with tc.tile_pool(name="p0ps", bufs=2, space="PSUM") as p0ps:
                for m in range(4):
                    ps = p0ps.tile([128, SL], F32, tag="mr")
                    for k in range(KT):
                        nc.tensor.matmul(ps[:], memt_sb[:, k, ds(m * 128, 128)],
                                         wr_sb[:, k, :],
                                         start=(k == 0), stop=(k == KT - 1))
                    nc.vector.tensor_copy(mr_sb[:, m, :], ps[:])

            # P2: the recurrence
            with tc.tile_pool(name="loop", bufs=3) as pool, \
                 tc.tile_pool(name="small", bufs=4) as sm, \
                 tc.tile_pool(name="cstate", bufs=2) as cpool2, \
                 tc.tile_pool(name="psS", bufs=2, space="PSUM") as psS, \
                 tc.tile_pool(name="psG", bufs=4, space="PSUM") as psG, \
                 tc.tile_pool(name="psB", bufs=1, space="PSUM") as psB, \
                 tc.tile_pool(name="dram", bufs=3, space="DRAM") as dram:

                gat = pool.tile([128, KT, B], F32R, tag="gat")
                nc.sync.dma_start(gat[:], h0t.ap().rearrange("k p b -> p k b"))

                def open_gates_psum(t):
                    """x-part + bias of step t's gate pre-activations; no
                    dependence on h_{t-1} -> fills the AG idle window."""
                    xts = pool.tile([128, KT, B], F32R, tag="xts")
                    nc.sync.dma_start(xts[:],
                                      xt_in.ap()[t].rearrange("k p b -> p k b"))
                    ps_g = psG.tile([B, SL], F32, tag="g")
                    for k in range(KT):
                        nc.tensor.matmul(ps_g[:], xts[:, k, :], wx_sb[:, k, :],
                                         start=(k == 0), stop=False)
                    nc.tensor.matmul(ps_g[:], ones_sb[:], biasv_sb[:],
                                     start=False, stop=False)
                    return ps_g

                LOOKAHEAD = 3
                psg_pending = {}
                for tt in range(min(LOOKAHEAD, T)):
                    psg_pending[tt] = open_gates_psum(tt)

                for t in range(T):
                    ps_g = psg_pending.pop(t)
                    xr_t = pool.tile([B, 128], F32, tag="xrt")
                    nc.sync.dma_start(xr_t[:], xres.ap()[t])

                    # attention over h_{t-1} (t=0 uses the r0 input instead)
                    if t > 0:
                        ps_s = psS.tile([B, N_MEM], F32, tag="s")
                        for k in range(KT):
                            nc.tensor.matmul(ps_s[:], gat[:, k, :],
                                             memt_sb[:, k, :],
                                             start=(k == 0), stop=(k == KT - 1))
                        negmax = sm.tile([B, 1], F32, tag="negmax")
                        nc.vector.tensor_reduce(negmax[:], ps_s[:],
                                                axis=mybir.AxisListType.X,
                                                op=ALU.max, negate=True)
                        e_sb = pool.tile([B, N_MEM], F32, tag="e")
                        sumexp = sm.tile([B, 1], F32, tag="sumexp")
                        nc.scalar.activation(e_sb[:], ps_s[:], AF.Exp,
                                             bias=negmax[:], scale=1.0,
                                             accum_out=sumexp[:])
                        rec = sm.tile([B, 1], F32, tag="rec")
                        nc.vector.reciprocal(rec[:], sumexp[:])
                        e_r = pool.tile([B, N_MEM], F32R, tag="er")
                        nc.vector.tensor_scalar(e_r[:], e_sb[:], rec[:], None,
                                                op0=ALU.mult)
                        ps_e = psB.tile([128, 4, B], F32R, tag="et")
                        for j in range(4):
                            nc.tensor.transpose(ps_e[:, j, :],
                                                e_r[:, ds(j * 128, 128)],
                                                ident_r[:])
                        et_sb = pool.tile([128, 4, B], F32R, tag="ets")
                        nc.vector.tensor_copy(et_sb[:], ps_e[:])

                    for k in range(KT):
                        nc.tensor.matmul(ps_g[:], gat[:, k, :], wh_sb[:, k, :],
                                         start=False, stop=False)
                    if t == 0:
                        for k in range(KT):
                            nc.tensor.matmul(ps_g[:], r0t_sb[:, k, :],
                                             wr_sb[:, k, :],
                                             start=False, stop=(k == KT - 1))
                    else:
                        for j in range(4):
                            nc.tensor.matmul(ps_g[:], et_sb[:, j, :],
                                             mr_sb[:, j, :],
                                             start=False, stop=(j == 3))

                    # nonlinearities straight out of PSUM; col order [i|f|o|g]
                    iof = pool.tile([B, 384], F32, tag="iof")
                    nc.vector.tensor_scalar(iof[:], ps_g[:, 0:384], 0.0, 1.0,
                                            op0=ALU.max, op1=ALU.min)
                    gg = pool.tile([B, 128], F32, tag="gg")
                    nc.scalar.activation(gg[:], ps_g[:, ds(384, 128)], AF.Tanh)

                    ig = sm.tile([B, 128], F32, tag="ig")
                    nc.vector.tensor_tensor(ig[:], iof[:, 0:128], gg[:],
                                            op=ALU.mult)
                    c_new = cpool2.tile([B, 128], F32, tag="c")
                    nc.vector.tensor_tensor(c_new[:], iof[:, ds(128, 128)],
                                            c_st[:], op=ALU.mult)
                    nc.vector.tensor_tensor(c_new[:], c_new[:], ig[:],
                                            op=ALU.add)
                    c_st = c_new
                    h_t = pool.tile([B, 128], F32, tag="h")
                    nc.vector.tensor_tensor(h_t[:], iof[:, ds(256, 128)],
                                            c_new[:], op=ALU.mult)
                    nc.vector.tensor_tensor(h_t[:], h_t[:], xr_t[:],
                                            op=ALU.add)
                    nc.sync.dma_start(hout.ap()[t], h_t[:])

                    if t < T - 1:
                        ps_h = psB.tile([128, B], F32, tag="ht")
                        nc.tensor.transpose(ps_h[:], h_t[:], ident_f[:])
                        ht_sb = pool.tile([128, B], F32R, tag="hts")
                        nc.vector.tensor_copy(ht_sb[:], ps_h[:])
                        ib = dram.tile([128, B], F32R, tag="agin")
                        nc.sync.dma_start(ib[:], ht_sb[:])
                        ob = dram.tile([N_CORES * 128, B], F32R, tag="agout")
                        nc.gpsimd.collective_compute(
                            "AllGather", ALU.bypass,
                            ins=[ib[:]], outs=[ob[:]],
                            replica_groups=[list(range(N_CORES))],
                        )
                        # x-part of step t+2 (2-step lookahead): sits before
                        # scores(t+1) in the static PE order, so it executes
                        # during the collective window.
                        if t + LOOKAHEAD < T:
                            psg_pending[t + LOOKAHEAD] = open_gates_psum(t + LOOKAHEAD)
                        gat = pool.tile([128, KT, B], F32R, tag="gat")
                        obv = ob.rearrange("(s p) b -> p s b", p=128)
                        nc.sync.dma_start(gat[:, 0:3, :], obv[:, 0:3, :])
                        nc.scalar.dma_start(gat[:, 3:6, :], obv[:, 3:6, :])
                        nc.gpsimd.dma_start(gat[:, 6:8, :], obv[:, 6:8, :])

    nc.compile()
    return nc


def _prep_inputs(x, h0, r0, c0, memory, WI, bI, WF, bF, WC, bC, WO, bO, T):
    x = np.ascontiguousarray(np.asarray(x, np.float32)[:, :T])
    Ws = {"i": np.asarray(WI, np.float32), "f": np.asarray(WF, np.float32),
          "o": np.asarray(WO, np.float32), "g": np.asarray(WC, np.float32)}
    bs = {"i": np.asarray(bI, np.float32), "f": np.asarray(bF, np.float32),
          "o": np.asarray(bO, np.float32), "g": np.asarray(bC, np.float32)}
    memory = np.asarray(memory, np.float32)
    in_maps = []
    xt_full = np.ascontiguousarray(x.transpose(1, 2, 0).reshape(T, KT, 128, B))
    memt = memory.T.reshape(KT, 128, N_MEM).copy()
    h0t = np.asarray(h0, np.float32).T.reshape(KT, 128, B).copy()
    r0t = np.asarray(r0, np.float32).T.reshape(KT, 128, B).copy()
    ones = np.ones((1, B), np.float32)
    for c in range(N_CORES):
        sl = slice(c * 128, (c + 1) * 128)
        w_slices, b_parts = [], []
        for gname in ["i", "f", "o", "g"]:
            wg = Ws[gname][:, sl]
            bg = bs[gname][sl]
            if gname != "g":
                wg = wg * 0.2
                bg = bg * 0.2 + 0.5
            w_slices.append(wg)
            b_parts.append(bg)
        wfull = np.concatenate(w_slices, axis=1)  # [3072, 512]
        bias = np.concatenate(b_parts)
        in_maps.append({
            "wh": np.ascontiguousarray(wfull[0:H].reshape(KT, 128, SL)),
            "wr": np.ascontiguousarray(wfull[H:2 * H].reshape(KT, 128, SL)),
            "wx": np.ascontiguousarray(wfull[2 * H:3 * H].reshape(KT, 128, SL)),
            "memt": memt,
            "xt": xt_full,
            "xres": np.ascontiguousarray(x[:, :, sl].transpose(1, 0, 2)),
            "biasv": bias[None, :],
            "ones": ones,
            "h0t": h0t,
            "r0t": r0t,
            "c0": np.ascontiguousarray(np.asarray(c0, np.float32)[:, sl]),
        })
    return in_maps


def kernel(**inputs):
    T = int(inputs["x"].shape[1])
    if T not in _cache:
        _cache[T] = _build(T)
    nc = _cache[T]
    in_maps = _prep_inputs(T=T, **inputs)
    res = run_bass_kernel_spmd(nc, in_maps, core_ids=list(range(N_CORES)))
    hs = np.concatenate([res.results[c]["hout"] for c in range(N_CORES)],
                        axis=2)
    return np.ascontiguousarray(hs.transpose(1, 0, 2)).astype(np.float32)
